# revision 50
# baseline (speedup 1.0000x reference)
"""DCT-attention Trainium2 kernel (8 NeuronCores, data-parallel over batch).

Reference math (per b, h):
    Qd = dct @ (Q*s);  Kd = dct @ (K*s*mask);  Vd = dct @ (V*mask)   # [M,D]
    E  = Qd @ Kd^T;  P = softmax(E, axis=-1);  ctx = P @ Vd          # [M,D]
    x  = dct^T @ ctx                                                 # [N,D]
with B,H,N,D = 8,12,2048,64, M = 256, s = D**-0.25.

Sharding: batch b -> core b (8 cores). Host folds scale into Q/K and mask into
K/V, transposes to [N, H*D], bf16-casts; matmuls run bf16 -> fp32 PSUM; output
returns bf16 and is cast to f32 on the host.

DCT parity symmetry: dct[k, N-1-i] = (-1)^k dct[k, i].  The host uploads X
folded as [A; B] with A = X[:N/2] + reverse(X[N/2:]), B = X[:N/2] -
reverse(X[N/2:]), and the M axis reordered to [even k | odd k]; every
projection then contracts over N/2 instead of N.

Schedule per core (input DMA on the sync HWDGE queue; PE chases arrivals):
  DMA:  dctT | q | k | v (ODD-fold chunks first) | dct
  PE:   Q-proj A/B, K-proj A (chunk-major, scoped 7-bank pool with HAM
        warm-up junk matmuls) -> even-k energy pairs in the k-odd DMA
        shadow -> K-proj B (head-pair-major, 2 banks) + odd-k energy
        pairs -> V-proj odd -> V-proj even + phase-B kb1
        pre-accumulation (24 open groups, 12 heads packed per 2-bank
        tile at a 7/5 bank split) -> kb0 + batched broadcast-mul
        normalize -> inverse-DCT sweep (full-width rows, out piece DMA
        every 2 row-blocks).
Each energy pair is two row-tiled 64-contraction matmuls into SEPARATE
banks of a [128,2,512] tile (concurrent row-tiles must not share a PSUM
bank), followed by one 512-wide exp.
"""

import numpy as np
import ml_dtypes

B, H, N, D = 8, 12, 2048, 64
M = 256
HD = H * D          # 768
NH = N // 2         # 1024 folded length
NCH = NH // 128     # 8 folded chunks per parity phase
MB = M // 128       # 2 m-blocks (even ks | odd ks)
HP = H // 2         # 6 head-pairs
FSPLIT = 2          # HD split for <=512-wide psum
FW = HD // FSPLIT   # 384
VW = D + 1          # 65: Vd columns + ones column
NB = NH // 128      # 8 output row-blocks per half
PCH = 2             # row-blocks per out piece

_BF16 = ml_dtypes.bfloat16
_CACHE = {}

# q is split finer at the head of the stream so the PE starts sooner.
Q_PIECES = [2, 2, 4, 4, 4]
KV_PIECES = [4, 4, 4, 4]


def build_nc():
    import concourse.bacc as bacc
    import concourse.mybir as mybir
    import concourse.tile as tile
    from concourse.bass import AP as BassAP
    from contextlib import ExitStack

    BF = mybir.dt.bfloat16
    F32 = mybir.dt.float32
    EXP = mybir.ActivationFunctionType.Exp
    COPY = mybir.ActivationFunctionType.Copy

    # All DRAM tensors are host-pre-swizzled partition-major ([128, ...])
    # so every DMA moves large contiguous per-partition blocks.
    nc = bacc.Bacc()
    q_d = nc.declare_dram_parameter("q", [128, 2 * NCH, HD], BF, isOutput=False)
    k_d = nc.declare_dram_parameter("k", [128, 2 * NCH, HD], BF, isOutput=False)
    v_d = nc.declare_dram_parameter("v", [128, 2 * NCH, HD], BF, isOutput=False)
    dctT_d = nc.declare_dram_parameter("dctT", [128, NCH, M], BF, isOutput=False)
    dct_d = nc.declare_dram_parameter("dct", [128, MB, NH], BF, isOutput=False)
    out_d = nc.declare_dram_parameter("out", [128, 2 * NB, HD], BF, isOutput=True)

    q_r = q_d.ap()
    k_r = k_d.ap()
    v_r = v_d.ap()
    dctT_r = dctT_d.ap()
    dct_r = dct_d.ap()
    out_r = out_d.ap()

    def bcast(ap, n):  # [128, m] -> [128, m, n] with stride-0 inner dim
        return BassAP(ap.tensor, ap.offset, [*ap.ap, [0, n]])

    with ExitStack() as ctx:
        tc = ctx.enter_context(tile.TileContext(nc))
        consts = ctx.enter_context(tc.tile_pool(name="consts", bufs=1))
        xin = ctx.enter_context(tc.tile_pool(name="xin", bufs=1))
        proj = ctx.enter_context(tc.tile_pool(name="proj", bufs=1))
        pbuf = ctx.enter_context(tc.tile_pool(name="pbuf", bufs=1))
        rbuf = ctx.enter_context(tc.tile_pool(name="rbuf", bufs=4))
        ostage = ctx.enter_context(tc.tile_pool(name="ostage", bufs=4))

        # ---- DMA stream (sync queue; order == consumption order) ----
        dctT_sb = consts.tile([128, NCH, M], BF)       # [n'-part, chunk, m]
        nc.sync.dma_start(dctT_sb[:], dctT_r)

        def stream(name, src_r, pieces, chunk_order=None):
            chunk_map = {}
            c0 = 0
            for pi, nch in enumerate(pieces):
                t = xin.tile([128, nch, HD], BF, tag=f"{name}{pi}")
                cs = [
                    (chunk_order[c0 + j] if chunk_order else c0 + j)
                    for j in range(nch)
                ]
                nc.sync.dma_start(t[:], src_r[:, cs[0]:cs[0] + nch, :])
                for j, c in enumerate(cs):
                    chunk_map[c] = (t, j)
                c0 += nch
            return chunk_map

        q_t = stream("q", q_r, Q_PIECES)   # chunks 0..7 = A-fold, 8..15 = B
        k_t = stream("k", k_r, KV_PIECES)
        # v arrives ODD-fold (B) chunks first so phase-B kb1 can
        # pre-accumulate during the v-even window.
        v_order = list(range(NCH, 2 * NCH)) + list(range(NCH))
        v_t = stream("v", v_r, KV_PIECES, chunk_order=v_order)

        dct_sb = consts.tile([128, MB, NH], BF)        # [m-part, m-block, n']
        nc.sync.dma_start(dct_sb[:], dct_r)

        # ---- persistent intermediates ----
        qdT_sb = proj.tile([128, HP, M], BF, tag="qdT")   # [2-head d, pair, m]
        kdT_sb = proj.tile([128, HP, M], BF, tag="kdT")
        vd_sb = proj.tile([128, MB, H, VW], BF, tag="vd")  # [k-part, kb, h, d+1]
        ctx_sb = proj.tile([128, MB, HD], BF, tag="ctx")   # [m-part, mb, h*d]
        ctxn_sb = proj.tile([128, HD], BF, tag="ctxn")     # -ctx odd block
        nc.vector.memset(vd_sb[:, :, :, D:VW], 1.0)
        ebias = consts.tile([128, 1], F32)
        nc.vector.memset(ebias[:], -4.0)
        # Dummy exp: hoists the ~1.3us ACT_TABLE_LOAD off the first real
        # exp's critical path (the table loads before the first ACTIVATE
        # in program order, which would otherwise be exp #1 of the chain).
        escr = consts.tile([128, 1], F32)
        nc.scalar.activation(escr[:], ebias[:], EXP)

        def xc(cm, c):  # folded chunk c (0..15), [128, HD]
            t, j = cm[c]
            return t[:, j, :]

        # ---- Q-proj A/B + K-proj A: chunk-major in a scoped 7-bank pool ---
        with tc.tile_pool(name="psP", bufs=7, space="PSUM") as psP:
            # HAM warm-up: tiny junk matmuls fill the PE's DMA-wait gaps
            # during Q-proj A so the clock gate reaches 2.4 GHz before the
            # compute-dense phases.  They write a dead 1-bank tile.
            wt = psP.tile([128, 64], F32, tag="P", name="warm")

            def junk(n):
                for _ in range(n):
                    nc.tensor.matmul(
                        wt[0:64, :],
                        lhsT=dctT_sb[0:64, 0, 0:64],
                        rhs=dctT_sb[0:64, 0, 0:64],
                        start=True,
                        stop=True,
                    )

            def proj_phase(cm, dst_sb, par, warm=False):
                groups = [
                    psP.tile([128, 128], F32, tag="P", name=f"pg{par}{hp}")
                    for hp in range(HP)
                ]
                for c in range(NCH):
                    for hp in range(HP):
                        nc.tensor.matmul(
                            groups[hp][:],
                            lhsT=xc(cm, par * NCH + c)[:, hp * 128:(hp + 1) * 128],
                            rhs=dctT_sb[:, c, par * 128:(par + 1) * 128],
                            start=(c == 0),
                            stop=(c == NCH - 1),
                        )
                    if warm:
                        junk(3)
                for hp in range(HP):
                    nc.vector.tensor_copy(
                        dst_sb[:, hp, par * 128:(par + 1) * 128], groups[hp][:]
                    )

            junk(6)
            proj_phase(q_t, qdT_sb, 0, warm=True)
            proj_phase(q_t, qdT_sb, 1)
            proj_phase(k_t, kdT_sb, 0)

        psE = ctx.enter_context(tc.tile_pool(name="psE", bufs=2, space="PSUM"))
        psW = ctx.enter_context(tc.tile_pool(name="psW", bufs=1, space="PSUM"))
        wt2 = psW.tile([128, 64], F32, tag="W", name="warm2")

        def junk2(n):
            # HAM hold-open fill for the arrival-gated V window (the PE
            # otherwise idles long enough mid-kernel to re-throttle).
            for _ in range(n):
                nc.tensor.matmul(
                    wt2[0:64, :],
                    lhsT=dctT_sb[0:64, 0, 0:64],
                    rhs=dctT_sb[0:64, 0, 0:64],
                    start=True,
                    stop=True,
                )
        psA = ctx.enter_context(tc.tile_pool(name="psA", bufs=2, space="PSUM"))

        # ---- energy pair: 2 heads of one head-pair; separate banks --------
        p_tiles = [None] * HP   # [128, MB, 2, M] bf16 per head-pair

        def emit_energy_pair(hp, kb):
            if p_tiles[hp] is None:
                p_tiles[hp] = pbuf.tile(
                    [128, MB, 2, M], BF, tag=f"p{hp}", name=f"p{hp}"
                )
            # [128, 2, 512] fp32 = 2 banks; head j's energy lands in bank j.
            pe = psE.tile([128, 2, 2 * M], F32, tag="E", name=f"e{hp}{kb}")
            for j in range(2):
                nc.tensor.matmul(
                    pe[:, j, 0:M],
                    lhsT=kdT_sb[64 * j:64 * j + 64, hp, kb * 128:(kb + 1) * 128],
                    rhs=qdT_sb[64 * j:64 * j + 64, hp, :],
                    start=True,
                    stop=True,
                )
            # P^T[k-block, m] = exp(E^T - 4); the -4 cancels in the
            # normalization and guards exp overflow for outlier logits.
            nc.scalar.activation(
                p_tiles[hp][:, kb, :, :], pe[:, :, 0:M], EXP, bias=ebias[:]
            )

        # even-k pairs run in the k-odd DMA shadow (kdT even + qdT ready)
        for hp in range(HP):
            emit_energy_pair(hp, 0)

        # ---- K-proj B: head-pair-major (2 banks) + odd-k pairs ------------
        for hp in range(HP):
            g = psA.tile([128, 128], F32, tag="A", name=f"kg{hp}")
            for c in range(NCH):
                nc.tensor.matmul(
                    g[:],
                    lhsT=xc(k_t, NCH + c)[:, hp * 128:(hp + 1) * 128],
                    rhs=dctT_sb[:, c, 128:256],
                    start=(c == 0),
                    stop=(c == NCH - 1),
                )
            nc.vector.tensor_copy(kdT_sb[:, hp, 128:256], g[:])
            emit_energy_pair(hp, 1)

        # ---- phase-B partial sums: one [128, 2, 512] tile (2 banks) per
        # m-block holds all 12 heads: bank 0 carries heads 0-6 at 65-col
        # pitch, bank 1 heads 7-11 (slices never straddle a bank).
        pc_tiles = {}

        def pc_of(h, mb):
            if mb not in pc_tiles:
                pc_tiles[mb] = psE.tile(
                    [128, 2, 512], F32, tag="E", name=f"pc{mb}"
                )
            b, j = divmod(h, 7)
            return pc_tiles[mb][:, b, j * VW:j * VW + VW]

        def phase_b_mm(h, mb, kb):
            # One accumulation group per PSUM bank: start=True zeroes the
            # whole 2KB zero-region, so only the bank's FIRST matmul (a kb1,
            # emitted during the v-even window) starts; the other slices'
            # first writes land on pending-zero bytes and overwrite.  Only
            # the bank's LAST matmul (a kb0) stops.
            hp, j = h // 2, h % 2
            nc.tensor.matmul(
                pc_of(h, mb),
                lhsT=p_tiles[hp][:, kb, j, mb * 128:(mb + 1) * 128],
                rhs=vd_sb[:, kb, h, :],
                start=(kb == 1 and h in (0, 7)),
                stop=(kb == 0 and h in (6, 11)),
            )

        # ---- V-proj: odd parity first -------------------------------------
        def evict_v(groups, par):
            for fs in range(FSPLIT):
                src = groups[fs][:].rearrange("p (h x) -> p h x", x=D)
                for half in range(2):
                    h0 = fs * 6 + half * 3
                    nc.vector.tensor_copy(
                        vd_sb[:, par, h0:h0 + 3, 0:D],
                        src[:, half * 3:half * 3 + 3, :],
                    )

        vgB = [
            psA.tile([128, FW], F32, tag="A", name=f"vgB{fs}")
            for fs in range(FSPLIT)
        ]
        for c in range(NCH):
            for fs in range(FSPLIT):
                nc.tensor.matmul(
                    vgB[fs][:],
                    lhsT=dctT_sb[:, c, 128:256],
                    rhs=xc(v_t, NCH + c)[:, fs * FW:(fs + 1) * FW],
                    start=(c == 0),
                    stop=(c == NCH - 1),
                )
            junk2(2)
        evict_v(vgB, 1)

        # ---- V-proj even + phase-B kb1 pre-accumulation -------------------
        vgA = [
            psA.tile([128, FW], F32, tag="A", name=f"vgA{fs}")
            for fs in range(FSPLIT)
        ]
        kb1q = [(h, mb) for mb in range(MB) for h in range(H)]
        for c in range(NCH):
            for fs in range(FSPLIT):
                nc.tensor.matmul(
                    vgA[fs][:],
                    lhsT=dctT_sb[:, c, 0:128],
                    rhs=xc(v_t, c)[:, fs * FW:(fs + 1) * FW],
                    start=(c == 0),
                    stop=(c == NCH - 1),
                )
            junk2(2)
            for _ in range(3):
                if kb1q:
                    phase_b_mm(*kb1q.pop(0), 1)
        while kb1q:
            phase_b_mm(*kb1q.pop(0), 1)
        evict_v(vgA, 0)

        # ---- phase-B kb0 + batched normalize (per pc bank) ----------------
        rsn_tiles = {}
        BANK_H = [(0, 7), (7, 5)]   # (first head, head count) per bank
        for mb in range(MB):
            for b, (h0, nh) in enumerate(BANK_H):
                for i in range(nh):
                    phase_b_mm(h0 + i, mb, 0)
                pcb = pc_tiles[mb][:, b, 0:nh * VW].rearrange(
                    "p (h x) -> p h x", x=VW
                )
                rs = rbuf.tile([128, nh], F32, tag=f"r{b}", name=f"r{mb}{b}")
                nc.vector.reciprocal(rs[:], pcb[:, :, D])
                dst = ctx_sb[:, mb, h0 * D:(h0 + nh) * D].rearrange(
                    "p (h x) -> p h x", x=D
                )
                nc.vector.tensor_mul(dst, pcb[:, :, 0:D], bcast(rs[:], D))
                if mb == 1:
                    rsn = rbuf.tile(
                        [128, nh], F32, tag=f"rn{b}", name=f"rn{b}"
                    )
                    nc.vector.tensor_scalar_mul(rsn[:], rs[:], -1.0)
                    rsn_tiles[b] = (rsn, pcb, h0, nh)
        # negated odd-k ctx for the reconstructed upper output half
        for b, (rsn, pcb, h0, nh) in rsn_tiles.items():
            dst = ctxn_sb[:, h0 * D:(h0 + nh) * D].rearrange(
                "p (h x) -> p h x", x=D
            )
            nc.vector.tensor_mul(dst, pcb[:, :, 0:D], bcast(rsn[:], D))

        # ---- inverse DCT: full-width row-blocks, out piece every 2 blocks -
        # half 0: x[0:1024] = even + odd contributions.
        # half 1: y[j] = x[N-1-j] = even - odd (host un-reverses rows 1024:,
        # so we accumulate with the negated odd-block ctx).
        for half in range(2):
            for pi in range(NB // PCH):
                ost = ostage.tile(
                    [128, PCH, HD], BF, tag=f"o{pi % 4}", name=f"o{half}{pi}"
                )
                for nbi in range(PCH):
                    nb = pi * PCH + nbi
                    px = [
                        (psA if fs == 0 else psE).tile(
                            [128, FW], F32, tag=("A" if fs == 0 else "E"),
                            name=f"x{half}{nb}{fs}",
                        )
                        for fs in range(FSPLIT)
                    ]
                    for fs in range(FSPLIT):
                        nc.tensor.matmul(
                            px[fs][:],
                            lhsT=dct_sb[:, 0, nb * 128:(nb + 1) * 128],
                            rhs=ctx_sb[:, 0, fs * FW:(fs + 1) * FW],
                            start=True,
                            stop=False,
                        )
                    for fs in range(FSPLIT):
                        odd_rhs = (
                            ctx_sb[:, 1, fs * FW:(fs + 1) * FW] if half == 0
                            else ctxn_sb[:, fs * FW:(fs + 1) * FW]
                        )
                        nc.tensor.matmul(
                            px[fs][:],
                            lhsT=dct_sb[:, 1, nb * 128:(nb + 1) * 128],
                            rhs=odd_rhs,
                            start=False,
                            stop=True,
                        )
                    for fs in range(FSPLIT):
                        dst = ost[:, nbi, fs * FW:(fs + 1) * FW]
                        if (nb + fs) % 2 == 0:
                            nc.vector.tensor_copy(dst, px[fs][:])
                        else:
                            nc.scalar.activation(dst, px[fs][:], COPY)
                nc.sync.dma_start(
                    out_r[:, half * NB + pi * PCH:half * NB + (pi + 1) * PCH, :],
                    ost[:],
                )

    nc.compile()
    return nc


def prep_in_maps(Q, K, V, mask, Q_dct):
    Q, K, V = np.asarray(Q), np.asarray(K), np.asarray(V)
    mask, Q_dct = np.asarray(mask), np.asarray(Q_dct)
    scale = np.float32(1.0 / np.sqrt(np.sqrt(np.float32(D))))
    m4 = mask.astype(np.float32)[:, None, :, None]        # [B,1,N,1]

    def fold(x):  # [B,N,HD] -> [A; B] along N
        lo, hi = x[:, :NH, :], x[:, NH:, :][:, ::-1, :]
        return np.concatenate([lo + hi, lo - hi], axis=1)

    qs = fold((Q.astype(np.float32) * scale).transpose(0, 2, 1, 3).reshape(B, N, HD))
    ks = fold((K.astype(np.float32) * scale * m4).transpose(0, 2, 1, 3).reshape(B, N, HD))
    vs = fold((V.astype(np.float32) * m4).transpose(0, 2, 1, 3).reshape(B, N, HD))
    def swz(x, nch):  # [B, nch*128, F] -> [B, 128, nch, F] partition-major
        return np.ascontiguousarray(
            x.reshape(B, nch, 128, -1).transpose(0, 2, 1, 3)
        ).astype(_BF16)

    qs, ks, vs = swz(qs, 2 * NCH), swz(ks, 2 * NCH), swz(vs, 2 * NCH)

    dct_f = Q_dct.astype(np.float32)
    perm = np.concatenate([np.arange(0, M, 2), np.arange(1, M, 2)])
    dct_p = dct_f[perm]                            # rows reordered [even|odd]
    dct = np.ascontiguousarray(                    # [128, MB, NH]
        dct_p[:, :NH].reshape(MB, 128, NH).transpose(1, 0, 2)
    ).astype(_BF16)
    dctT = np.ascontiguousarray(                   # [128, NCH, M]
        dct_p[:, :NH].T.reshape(NCH, 128, M).transpose(1, 0, 2)
    ).astype(_BF16)
    return [
        {"q": qs[b], "k": ks[b], "v": vs[b], "dctT": dctT, "dct": dct}
        for b in range(B)
    ]


def unswizzle_out(arr):
    """[128, 16, HD] partition-major device output -> [N, HD] rows, with the
    upper half un-reversed (device rows hold y[j] = x[N-1-j])."""
    x = np.asarray(arr).astype(np.float32)
    x = x.transpose(1, 0, 2).reshape(N, HD)
    x[NH:] = x[NH:][::-1]
    return x


def run(Q, K, V, mask, Q_dct, trace=False):
    from concourse.bass_utils import run_bass_kernel_spmd

    if "nc" not in _CACHE:
        _CACHE["nc"] = build_nc()
    nc = _CACHE["nc"]
    in_maps = prep_in_maps(Q, K, V, mask, Q_dct)
    res = run_bass_kernel_spmd(nc, in_maps, core_ids=list(range(B)), trace=trace)
    outs = np.stack([unswizzle_out(res.results[i]["out"]) for i in range(B)])
    x = outs.reshape(B, N, H, D).transpose(0, 2, 1, 3)
    return np.ascontiguousarray(x, dtype=np.float32), res


def kernel(Q, K, V, mask, Q_dct):
    x, _ = run(Q, K, V, mask, Q_dct, trace=False)
    return x
